# revision 16
# baseline (speedup 1.0000x reference)
"""Trainium2 Bass kernel for nn_Lorec (moe_routing LoRA-with-soft-routing).

Computation (per batch b):
  gate_b = softmax(MLP(LayerNorm(ctr[b])))                    [16]
  A_b[i,r] = sum_f Wa[r*4096+i, f] gate_b[f]                  [4096,16]
  B_b[r,o] = sum_f Wb[r*4096+o, f] gate_b[f]                  [16,4096]
  out[b] = (x[b] @ A_b) @ B_b * 2.0                           [2048,4096]

Sharding: data-parallel over bs=8 across 8 NeuronCores (one batch per core).
Gating is replicated on every core (tiny); each core selects its own batch's
gate row via a per-core one-hot input. Adapter weights replicated.

This version is tuned for the DMA roofline: all big tensors move as fp16
(x pre-transposed on the host so mm1 needs no PE transposes; y stored fp16
and upcast on the host). Per-core HBM traffic is 16 MiB x + 4 MiB W in,
16 MiB y out (~105 us at 360 GB/s), and the PE does ~72 us of work fully
overlapped with the DMA stream.

Device dataflow per core:
  - gating MLP + softmax on DVE/ACT with tiny PE transposes (fp32)
  - A/B generated on PE via the Kronecker trick: G = (I_16 kron gate) [128,32]
    fp16, A-chunk = WaP^T @ G (WaP = host-relaid Wa), B = G^T @ WbP.
  - mm1 over two seq halves: xaT[16,512][sb] += A_c^T @ xT_c with xT tiles
    DMA'd directly from the host-transposed x (fp16, 2 KB lines).
  - mm2: out[128s,512o] = xaT_t^T @ B (fp16), ACT/DVE copy to fp16 SBUF,
    DMA out. SCALING(2.0) folded into Wb on host.
"""

import sys

sys.path.insert(0, "/opt/trn_rl_repo")

import numpy as np

BS = 8
SEQ = 2048
IN = 4096
OUT = 4096
R = 16
CTR_OUT = 256
CTR_HID = 60
FD = 16  # FINAL_DIM
LN_EPS = 1e-5
SCALING = 2.0

P = 128
SBW = 512  # s-block width
NHALF = 2  # seq halves
SB_PER_HALF = 2  # s-blocks per half
NC_I = IN // P  # 32 i-chunks
NOB = OUT // 512  # 8 o-blocks

_COMPILED = None


def build_program():
    import concourse.bass as bass
    import concourse.mybir as mybir
    from concourse import bacc
    from concourse.masks import make_identity
    from concourse.tile import TileContext

    f32 = mybir.dt.float32
    f16 = mybir.dt.float16
    AX = mybir.AxisListType.X
    ALU = mybir.AluOpType
    ACTF = mybir.ActivationFunctionType

    nc = bacc.Bacc("TRN2", target_bir_lowering=False, debug=False, num_devices=BS)

    xt_d = nc.dram_tensor("xt", [IN, SEQ], f16, kind="ExternalInput").ap()
    ctr_d = nc.dram_tensor("ctr", [BS, CTR_OUT], f32, kind="ExternalInput").ap()
    gam_d = nc.dram_tensor("gam", [BS, CTR_OUT], f32, kind="ExternalInput").ap()
    bet_d = nc.dram_tensor("bet", [BS, CTR_OUT], f32, kind="ExternalInput").ap()
    w1t_d = nc.dram_tensor("w1t", [P, 2 * CTR_HID], f32, kind="ExternalInput").ap()
    b1_d = nc.dram_tensor("b1", [CTR_HID, 1], f32, kind="ExternalInput").ap()
    w2t_d = nc.dram_tensor("w2t", [CTR_HID, FD], f32, kind="ExternalInput").ap()
    b2_d = nc.dram_tensor("b2", [FD, 1], f32, kind="ExternalInput").ap()
    wap_d = nc.dram_tensor("wap", [P, 2 * IN], f16, kind="ExternalInput").ap()
    wbp_d = nc.dram_tensor("wbp", [P, 2 * OUT], f16, kind="ExternalInput").ap()
    sel_d = nc.dram_tensor("sel", [R, BS], f32, kind="ExternalInput").ap()
    y_d = nc.dram_tensor("y", [SEQ, OUT], f16, kind="ExternalOutput").ap()

    with TileContext(nc) as tc:
        with (
            tc.tile_pool(name="const", bufs=1) as const,
            tc.tile_pool(name="gp", bufs=1) as gp,
            tc.tile_pool(name="xpool", bufs=48) as xpool,
            tc.tile_pool(name="xapool", bufs=4) as xapool,
            tc.tile_pool(name="opool", bufs=6) as opool,
            tc.tile_pool(name="psg_pool", bufs=1, space="PSUM") as psg_pool,
            tc.tile_pool(name="psxa_pool", bufs=2, space="PSUM") as psxa_pool,
            tc.tile_pool(name="pso_pool", bufs=5, space="PSUM") as pso_pool,
        ):
            # ---- big weight streams first so A/B-gen can start early ----
            wap_t = gp.tile([P, 2 * IN], f16)
            nc.sync.dma_start(out=wap_t[:], in_=wap_d[:])
            wbp_t = gp.tile([P, 2 * OUT], f16)
            nc.sync.dma_start(out=wbp_t[:], in_=wbp_d[:])

            ident = const.tile([P, P], f32)
            make_identity(nc, ident)

            # ---- gating inputs ----
            ctr = gp.tile([BS, CTR_OUT], f32)
            gam = gp.tile([BS, CTR_OUT], f32)
            bet = gp.tile([BS, CTR_OUT], f32)
            w1t = gp.tile([P, 2 * CTR_HID], f32)
            b1 = gp.tile([CTR_HID, 1], f32)
            w2t = gp.tile([CTR_HID, FD], f32)
            b2 = gp.tile([FD, 1], f32)
            sel = gp.tile([R, BS], f32)
            for t, d in [
                (ctr, ctr_d), (gam, gam_d), (bet, bet_d), (w1t, w1t_d),
                (b1, b1_d), (w2t, w2t_d), (b2, b2_d), (sel, sel_d),
            ]:
                nc.gpsimd.dma_start(out=t[:], in_=d[:])
            eps_t = gp.tile([BS, 1], f32)
            nc.gpsimd.memset(eps_t[:], LN_EPS)
            ones16 = gp.tile([FD, 1], f32)
            nc.gpsimd.memset(ones16[:], 1.0)
            ones128 = gp.tile([1, P], f32)
            nc.gpsimd.memset(ones128[:], 1.0)

            # ---- LayerNorm on [8, 256]: DVE-fused, rstd = exp(-0.5 ln(var+eps))
            # (ln+exp live in the same ACT table set as copy/relu -> one load)
            svs = gp.tile([BS, 1], f32)
            sq = gp.tile([BS, CTR_OUT], f32)
            vs = gp.tile([BS, 1], f32)
            mean = gp.tile([BS, 1], f32)
            m2 = gp.tile([BS, 1], f32)
            var = gp.tile([BS, 1], f32)
            lnv = gp.tile([BS, 1], f32)
            rstd = gp.tile([BS, 1], f32)
            xh = gp.tile([BS, CTR_OUT], f32)
            hh = gp.tile([BS, CTR_OUT], f32)
            nc.vector.tensor_reduce(svs[:], ctr[:], axis=AX, op=ALU.add)
            nc.vector.tensor_mul(sq[:], ctr[:], ctr[:])
            nc.vector.tensor_reduce(vs[:], sq[:], axis=AX, op=ALU.add)
            nc.vector.tensor_scalar_mul(mean[:], svs[:], 1.0 / CTR_OUT)
            nc.vector.tensor_mul(m2[:], mean[:], mean[:])
            nc.vector.tensor_scalar(
                var[:], vs[:], 1.0 / CTR_OUT, m2[:], op0=ALU.mult, op1=ALU.subtract
            )
            nc.scalar.activation(lnv[:], var[:], ACTF.Ln, bias=eps_t[:])
            nc.scalar.activation(rstd[:], lnv[:], ACTF.Exp, scale=-0.5)
            nc.vector.tensor_scalar(
                xh[:], ctr[:], mean[:], rstd[:], op0=ALU.subtract, op1=ALU.mult
            )
            nc.vector.tensor_mul(hh[:], xh[:], gam[:])
            nc.vector.tensor_add(hh[:], hh[:], bet[:])

            # ---- hT [256->2x128, 8] via PE transposes into one PSUM tile ----
            hT = gp.tile([P, 2 * BS], f32)
            pt = psg_pool.tile([P, SBW], f32, tag="psg")
            for h in range(2):
                nc.tensor.transpose(
                    pt[:, h * BS : (h + 1) * BS], hh[:, h * P : (h + 1) * P],
                    ident[0:BS, 0:BS],
                )
            nc.scalar.copy(hT[:], pt[:, 0 : 2 * BS])

            # ---- h1T = relu(W1 @ h + b1) -> [60, 8] ----
            ph1 = psg_pool.tile([P, SBW], f32, tag="psg")
            for h in range(2):
                nc.tensor.matmul(
                    ph1[0:CTR_HID, 0:BS], w1t[:, h * CTR_HID : (h + 1) * CTR_HID],
                    hT[:, h * BS : (h + 1) * BS], start=(h == 0), stop=(h == 1),
                )
            h1T = gp.tile([CTR_HID, BS], f32)
            nc.scalar.activation(h1T[:], ph1[0:CTR_HID, 0:BS], ACTF.Relu, bias=b1[:])

            # ---- unnormalized transposed gate: g = exp(W2 @ h1 + b2) [16, 8]
            # (softmax denominator deferred: 1/s^2 is applied by the out copies)
            plog = psg_pool.tile([P, SBW], f32, tag="psg")
            nc.tensor.matmul(plog[0:FD, 0:BS], w2t[:], h1T[:], start=True, stop=True)
            exT = gp.tile([FD, BS], f32)
            nc.scalar.activation(exT[:], plog[0:FD, 0:BS], ACTF.Exp, bias=b2[:])

            # select own batch's column via one-hot rows
            gsel = gp.tile([FD, BS], f32)
            g_b = gp.tile([FD, 1], f32)
            nc.vector.tensor_mul(gsel[:], exT[:], sel[:])
            nc.vector.tensor_reduce(g_b[:], gsel[:], axis=AX, op=ALU.add)
            g_bh = gp.tile([FD, 1], f16)
            nc.scalar.copy(g_bh[:], g_b[:])

            # ---- G = I_16 kron g_b (fp16), layout [128, 2*16] ----
            # 16 tiny SBUF->SBUF DMAs spread over 3 queues so they don't serialize
            G = gp.tile([P, 2 * FD], f16)
            nc.gpsimd.memset(G[:], 0.0)
            dma_engines = [nc.gpsimd, nc.scalar]
            for r in range(FD):
                h = r // 8
                p0 = (r % 8) * 16
                dma_engines[r % 2].dma_start(
                    out=G[p0 : p0 + 16, h * FD + r : h * FD + r + 1],
                    in_=g_bh[0:16, 0:1],
                )

            # ---- normalization side-chain (off critical path): rs2b[p] = 1/s^2
            pssum = pso_pool.tile([P, 512], f32, tag="pso", name="pssum")
            nc.tensor.matmul(pssum[0:1, 0:1], ones16[:], g_b[:], start=True, stop=True)
            s_sb = gp.tile([1, 1], f32)
            nc.scalar.copy(s_sb[:], pssum[0:1, 0:1])
            rs = gp.tile([1, 1], f32)
            nc.vector.reciprocal(rs[:], s_sb[:])
            rs2 = gp.tile([1, 1], f32)
            nc.vector.tensor_mul(rs2[:], rs[:], rs[:])
            psb2 = pso_pool.tile([P, 512], f32, tag="pso", name="psb2")
            nc.tensor.matmul(psb2[:, 0:1], ones128[:], rs2[:], start=True, stop=True)
            rs2b = gp.tile([P, 1], f32)
            nc.scalar.copy(rs2b[:], psb2[:, 0:1])

            # ---- A-gen: A_sb[p, c*16+r] = A[c*128+p, r] ----
            # h-major (h=0 needs only G's first 8 columns, ready earlier) and
            # chunk-grouped so mm1's first chunks can start before all of
            # A-gen is drained.
            A_sb = gp.tile([P, NC_I * R], f16)
            for cg in range(4):
                psA = psg_pool.tile([P, SBW], f32, tag="psg", name=f"psA{cg}")
                for cc in range(8):
                    c = cg * 8 + cc
                    for h in range(2):
                        nc.tensor.matmul(
                            psA[:, cc * R : (cc + 1) * R],
                            wap_t[:, h * IN + c * P : h * IN + (c + 1) * P],
                            G[:, h * FD : (h + 1) * FD],
                            start=(h == 0), stop=(h == 1),
                        )
                nc.scalar.copy(
                    A_sb[:, cg * 8 * R : (cg + 1) * 8 * R], psA[:, 0 : 8 * R]
                )

            # ---- B-gen: B_sb [16, 4096] (fp16), h-accumulated in PSUM ----
            B_sb = gp.tile([FD, OUT], f16)
            for ob in range(NOB):
                psB = psg_pool.tile([P, SBW], f32, tag="psg", name=f"psB{ob}")
                for h in range(2):
                    nc.tensor.matmul(
                        psB[0:FD, :],
                        G[:, h * FD : (h + 1) * FD],
                        wbp_t[:, h * OUT + ob * 512 : h * OUT + (ob + 1) * 512],
                        start=(h == 0), stop=(h == 1),
                    )
                nc.vector.tensor_copy(B_sb[:, ob * 512 : (ob + 1) * 512], psB[0:FD, :])

            # ---- main loop: 4 seq quarters of 512, mm2(q) overlaps mm1(q+1)
            for q in range(4):
                psxa = psxa_pool.tile([FD, SBW], f32, tag="psxa", name=f"psxa_{q}")
                for c in range(NC_I):
                    xt_c = xpool.tile([P, SBW], f16, tag="xnat")
                    nc.sync.dma_start(
                        out=xt_c[:],
                        in_=xt_d[c * P : (c + 1) * P, q * SBW : (q + 1) * SBW],
                    )
                    nc.tensor.matmul(
                        psxa[:],
                        A_sb[:, c * R : (c + 1) * R],
                        xt_c[:],
                        start=(c == 0), stop=(c == NC_I - 1),
                    )

                xaT = xapool.tile([FD, SBW], f16, tag="xaT")
                nc.vector.tensor_copy(xaT[:], psxa[:])
                for t in range(4):
                    out_sb = opool.tile([P, OUT], f16, tag="osb")
                    for ob in range(NOB):
                        pso = pso_pool.tile([P, 512], f32, tag="pso")
                        nc.tensor.matmul(
                            pso[:],
                            xaT[:, t * P : (t + 1) * P],
                            B_sb[:, ob * 512 : (ob + 1) * 512],
                            start=True, stop=True,
                        )
                        # PSUM->SBUF drain also applies the deferred
                        # softmax normalization 1/s^2 (GPSIMD can't read
                        # PSUM, so split over ACT and DVE)
                        dst = out_sb[:, ob * 512 : (ob + 1) * 512]
                        if ob % 2 == 0:
                            nc.scalar.activation(dst, pso[:], ACTF.Copy, scale=rs2b[:])
                        else:
                            nc.vector.tensor_scalar(
                                dst, pso[:], rs2b[:], None, op0=ALU.mult
                            )
                    srow = (q * 4 + t) * P
                    nc.scalar.dma_start(
                        out=y_d[srow : srow + P, :],
                        in_=out_sb[:],
                    )

    nc.compile()
    return nc


def host_prep(inputs):
    """Build per-core and shared input arrays from the full problem inputs."""
    x = np.asarray(inputs["x"], np.float32)
    ctr = np.ascontiguousarray(np.asarray(inputs["ctr_hidden_states"], np.float32))
    gam = np.ascontiguousarray(
        np.tile(np.asarray(inputs["ln_gamma"], np.float32)[None, :], (BS, 1))
    )
    bet = np.ascontiguousarray(
        np.tile(np.asarray(inputs["ln_beta"], np.float32)[None, :], (BS, 1))
    )
    W1 = np.asarray(inputs["W1"], np.float32)
    w1t = np.ascontiguousarray(
        W1.T.reshape(2, P, CTR_HID).transpose(1, 0, 2).reshape(P, 2 * CTR_HID)
    )
    b1 = np.ascontiguousarray(np.asarray(inputs["b1"], np.float32).reshape(CTR_HID, 1))
    w2t = np.ascontiguousarray(np.asarray(inputs["W2"], np.float32).T)
    b2 = np.ascontiguousarray(np.asarray(inputs["b2"], np.float32).reshape(FD, 1))
    Wa = np.asarray(inputs["Wa"], np.float32)
    WaP = Wa.reshape(R, IN, FD).transpose(0, 2, 1).reshape(R * FD, IN)
    wap = np.ascontiguousarray(
        WaP.reshape(2, P, IN).transpose(1, 0, 2).reshape(P, 2 * IN)
    ).astype(np.float16)
    Wb = np.asarray(inputs["Wb"], np.float32) * SCALING
    WbP = Wb.reshape(R, OUT, FD).transpose(0, 2, 1).reshape(R * FD, OUT)
    wbp = np.ascontiguousarray(
        WbP.reshape(2, P, OUT).transpose(1, 0, 2).reshape(P, 2 * OUT)
    ).astype(np.float16)

    shared = dict(
        ctr=ctr, gam=gam, bet=bet, w1t=w1t, b1=b1, w2t=w2t, b2=b2, wap=wap, wbp=wbp
    )
    in_maps = []
    for c in range(BS):
        onehot = np.zeros((BS,), np.float32)
        onehot[c] = 1.0
        sel = np.ascontiguousarray(np.tile(onehot[None, :], (R, 1)))
        m = dict(shared)
        m["sel"] = sel
        m["xt"] = np.ascontiguousarray(np.asarray(x[c], np.float16).T)
        in_maps.append(m)
    return in_maps


def get_compiled():
    global _COMPILED
    if _COMPILED is None:
        _COMPILED = build_program()
    return _COMPILED


def run(inputs, trace=False):
    from concourse.bass_utils import run_bass_kernel_spmd

    nc = get_compiled()
    in_maps = host_prep(inputs)
    res = run_bass_kernel_spmd(nc, in_maps, list(range(BS)), trace=trace)
    out = np.stack(
        [np.asarray(res.results[c]["y"], np.float32) for c in range(BS)], axis=0
    )
    return out, res


def kernel(**inputs) -> np.ndarray:
    out, _ = run(inputs, trace=False)
    return out


# revision 19
# speedup vs baseline: 1.3177x; 1.3177x over previous
"""Trainium2 Bass kernel for nn_Lorec (moe_routing LoRA-with-soft-routing).

Computation (per batch b):
  gate_b = softmax(MLP(LayerNorm(ctr[b])))                    [16]
  A_b[i,r] = sum_f Wa[r*4096+i, f] gate_b[f]                  [4096,16]
  B_b[r,o] = sum_f Wb[r*4096+o, f] gate_b[f]                  [16,4096]
  out[b] = (x[b] @ A_b) @ B_b * 2.0                           [2048,4096]

Sharding: data-parallel over bs=8 across 8 NeuronCores (one batch per core).
Gating is replicated on every core (tiny); each core selects its own batch's
gate row via a per-core one-hot input. Adapter weights replicated.

This version is tuned for the DMA roofline: all big tensors move as fp16
(x pre-transposed on the host so mm1 needs no PE transposes; y stored fp16
and upcast on the host). Per-core HBM traffic is 16 MiB x + 4 MiB W in,
16 MiB y out (~105 us at 360 GB/s), and the PE does ~72 us of work fully
overlapped with the DMA stream.

Device dataflow per core:
  - gating MLP + softmax on DVE/ACT with tiny PE transposes (fp32)
  - A/B generated on PE via the Kronecker trick: G = (I_16 kron gate) [128,32]
    fp16, A-chunk = WaP^T @ G (WaP = host-relaid Wa), B = G^T @ WbP.
  - mm1 over two seq halves: xaT[16,512][sb] += A_c^T @ xT_c with xT tiles
    DMA'd directly from the host-transposed x (fp16, 2 KB lines).
  - mm2: out[128s,512o] = xaT_t^T @ B (fp16), ACT/DVE copy to fp16 SBUF,
    DMA out. SCALING(2.0) folded into Wb on host.
"""

import sys

sys.path.insert(0, "/opt/trn_rl_repo")

import numpy as np

BS = 8
SEQ = 2048
IN = 4096
OUT = 4096
R = 16
CTR_OUT = 256
CTR_HID = 60
FD = 16  # FINAL_DIM
LN_EPS = 1e-5
SCALING = 2.0

P = 128
SBW = 512  # s-block width
NHALF = 2  # seq halves
SB_PER_HALF = 2  # s-blocks per half
NC_I = IN // P  # 32 i-chunks
NOB = OUT // 512  # 8 o-blocks

_COMPILED = None


def build_program():
    import concourse.bass as bass
    import concourse.mybir as mybir
    from concourse import bacc
    from concourse.masks import make_identity
    from concourse.tile import TileContext

    f32 = mybir.dt.float32
    f16 = mybir.dt.float16
    AX = mybir.AxisListType.X
    ALU = mybir.AluOpType
    ACTF = mybir.ActivationFunctionType

    nc = bacc.Bacc("TRN2", target_bir_lowering=False, debug=False, num_devices=BS)

    xt_d = nc.dram_tensor("xt", [IN, SEQ], f16, kind="ExternalInput").ap()
    ctr_d = nc.dram_tensor("ctr", [BS, CTR_OUT], f32, kind="ExternalInput").ap()
    gam_d = nc.dram_tensor("gam", [BS, CTR_OUT], f32, kind="ExternalInput").ap()
    bet_d = nc.dram_tensor("bet", [BS, CTR_OUT], f32, kind="ExternalInput").ap()
    w1t_d = nc.dram_tensor("w1t", [P, 2 * CTR_HID], f32, kind="ExternalInput").ap()
    b1_d = nc.dram_tensor("b1", [CTR_HID, 1], f32, kind="ExternalInput").ap()
    w2t_d = nc.dram_tensor("w2t", [CTR_HID, FD], f32, kind="ExternalInput").ap()
    b2_d = nc.dram_tensor("b2", [FD, 1], f32, kind="ExternalInput").ap()
    wap_d = nc.dram_tensor("wap", [P, 2 * IN], f16, kind="ExternalInput").ap()
    wbp_d = nc.dram_tensor("wbp", [P, 2 * OUT], f16, kind="ExternalInput").ap()
    sel_d = nc.dram_tensor("sel", [R, BS], f32, kind="ExternalInput").ap()
    y_d = nc.dram_tensor("y", [SEQ, OUT], f16, kind="ExternalOutput").ap()

    with TileContext(nc) as tc:
        with (
            tc.tile_pool(name="const", bufs=1) as const,
            tc.tile_pool(name="gp", bufs=1) as gp,
            tc.tile_pool(name="xpool", bufs=16) as xpool,
            tc.tile_pool(name="xapool", bufs=4) as xapool,
            tc.tile_pool(name="opool", bufs=6) as opool,
            tc.tile_pool(name="psg_pool", bufs=1, space="PSUM") as psg_pool,
            tc.tile_pool(name="psxa_pool", bufs=2, space="PSUM") as psxa_pool,
            tc.tile_pool(name="pso_pool", bufs=5, space="PSUM") as pso_pool,
        ):
            # ---- big weight streams first so A/B-gen can start early ----
            wap_t = gp.tile([P, 2 * IN], f16)
            nc.sync.dma_start(out=wap_t[:], in_=wap_d[:])
            wbp_t = gp.tile([P, 2 * OUT], f16)
            nc.sync.dma_start(out=wbp_t[:], in_=wbp_d[:])

            ident = const.tile([P, P], f32)
            make_identity(nc, ident)

            # ---- gating inputs ----
            ctr = gp.tile([BS, CTR_OUT], f32)
            gam = gp.tile([BS, CTR_OUT], f32)
            bet = gp.tile([BS, CTR_OUT], f32)
            w1t = gp.tile([P, 2 * CTR_HID], f32)
            b1 = gp.tile([CTR_HID, 1], f32)
            w2t = gp.tile([CTR_HID, FD], f32)
            b2 = gp.tile([FD, 1], f32)
            sel = gp.tile([R, BS], f32)
            for t, d in [
                (ctr, ctr_d), (gam, gam_d), (bet, bet_d), (w1t, w1t_d),
                (b1, b1_d), (w2t, w2t_d), (b2, b2_d), (sel, sel_d),
            ]:
                nc.gpsimd.dma_start(out=t[:], in_=d[:])
            eps_t = gp.tile([BS, 1], f32)
            nc.gpsimd.memset(eps_t[:], LN_EPS)
            ones16 = gp.tile([FD, 1], f32)
            nc.gpsimd.memset(ones16[:], 1.0)
            ones128 = gp.tile([1, P], f32)
            nc.gpsimd.memset(ones128[:], 1.0)

            # ---- LayerNorm on [8, 256]: DVE-fused, rstd = exp(-0.5 ln(var+eps))
            # (ln+exp live in the same ACT table set as copy/relu -> one load)
            svs = gp.tile([BS, 1], f32)
            sq = gp.tile([BS, CTR_OUT], f32)
            vs = gp.tile([BS, 1], f32)
            mean = gp.tile([BS, 1], f32)
            m2 = gp.tile([BS, 1], f32)
            var = gp.tile([BS, 1], f32)
            lnv = gp.tile([BS, 1], f32)
            rstd = gp.tile([BS, 1], f32)
            xh = gp.tile([BS, CTR_OUT], f32)
            hh = gp.tile([BS, CTR_OUT], f32)
            nc.vector.tensor_reduce(svs[:], ctr[:], axis=AX, op=ALU.add)
            nc.vector.tensor_mul(sq[:], ctr[:], ctr[:])
            nc.vector.tensor_reduce(vs[:], sq[:], axis=AX, op=ALU.add)
            nc.vector.tensor_scalar_mul(mean[:], svs[:], 1.0 / CTR_OUT)
            nc.vector.tensor_mul(m2[:], mean[:], mean[:])
            nc.vector.tensor_scalar(
                var[:], vs[:], 1.0 / CTR_OUT, m2[:], op0=ALU.mult, op1=ALU.subtract
            )
            nc.scalar.activation(lnv[:], var[:], ACTF.Ln, bias=eps_t[:])
            nc.scalar.activation(rstd[:], lnv[:], ACTF.Exp, scale=-0.5)
            nc.vector.tensor_scalar(
                xh[:], ctr[:], mean[:], rstd[:], op0=ALU.subtract, op1=ALU.mult
            )
            nc.vector.tensor_mul(hh[:], xh[:], gam[:])
            nc.vector.tensor_add(hh[:], hh[:], bet[:])

            # ---- hT [256->2x128, 8] via PE transposes into one PSUM tile ----
            hT = gp.tile([P, 2 * BS], f32)
            pt = psg_pool.tile([P, SBW], f32, tag="psg")
            for h in range(2):
                nc.tensor.transpose(
                    pt[:, h * BS : (h + 1) * BS], hh[:, h * P : (h + 1) * P],
                    ident[0:BS, 0:BS],
                )
            nc.scalar.copy(hT[:], pt[:, 0 : 2 * BS])

            # ---- h1T = relu(W1 @ h + b1) -> [60, 8] ----
            ph1 = psg_pool.tile([P, SBW], f32, tag="psg")
            for h in range(2):
                nc.tensor.matmul(
                    ph1[0:CTR_HID, 0:BS], w1t[:, h * CTR_HID : (h + 1) * CTR_HID],
                    hT[:, h * BS : (h + 1) * BS], start=(h == 0), stop=(h == 1),
                )
            h1T = gp.tile([CTR_HID, BS], f32)
            nc.scalar.activation(h1T[:], ph1[0:CTR_HID, 0:BS], ACTF.Relu, bias=b1[:])

            # ---- unnormalized transposed gate: g = exp(W2 @ h1 + b2) [16, 8]
            # (softmax denominator deferred: 1/s^2 is applied by the out copies)
            plog = psg_pool.tile([P, SBW], f32, tag="psg")
            nc.tensor.matmul(plog[0:FD, 0:BS], w2t[:], h1T[:], start=True, stop=True)
            exT = gp.tile([FD, BS], f32)
            nc.scalar.activation(exT[:], plog[0:FD, 0:BS], ACTF.Exp, bias=b2[:])

            # select own batch's column via one-hot rows
            gsel = gp.tile([FD, BS], f32)
            g_b = gp.tile([FD, 1], f32)
            nc.vector.tensor_mul(gsel[:], exT[:], sel[:])
            nc.vector.tensor_reduce(g_b[:], gsel[:], axis=AX, op=ALU.add)
            g_bh = gp.tile([FD, 1], f16)
            nc.scalar.copy(g_bh[:], g_b[:])

            # ---- G = I_16 kron g_b (fp16), layout [128, 2*16] ----
            # 16 tiny SBUF->SBUF DMAs spread over 3 queues so they don't serialize
            G = gp.tile([P, 2 * FD], f16)
            nc.gpsimd.memset(G[:], 0.0)
            dma_engines = [nc.gpsimd, nc.scalar]
            for r in range(FD):
                h = r // 8
                p0 = (r % 8) * 16
                dma_engines[r % 2].dma_start(
                    out=G[p0 : p0 + 16, h * FD + r : h * FD + r + 1],
                    in_=g_bh[0:16, 0:1],
                )

            # ---- normalization side-chain (off critical path): rs2b[p] = 1/s^2
            pssum = pso_pool.tile([P, 512], f32, tag="pso", name="pssum")
            nc.tensor.matmul(pssum[0:1, 0:1], ones16[:], g_b[:], start=True, stop=True)
            s_sb = gp.tile([1, 1], f32)
            nc.scalar.copy(s_sb[:], pssum[0:1, 0:1])
            rs = gp.tile([1, 1], f32)
            nc.vector.reciprocal(rs[:], s_sb[:])
            rs2 = gp.tile([1, 1], f32)
            nc.vector.tensor_mul(rs2[:], rs[:], rs[:])
            psb2 = pso_pool.tile([P, 512], f32, tag="pso", name="psb2")
            nc.tensor.matmul(psb2[:, 0:1], ones128[:], rs2[:], start=True, stop=True)
            rs2b = gp.tile([P, 1], f32)
            nc.scalar.copy(rs2b[:], psb2[:, 0:1])

            # ---- A-gen: A_sb[p, c*16+r] = A[c*128+p, r] ----
            # h-major (h=0 needs only G's first 8 columns, ready earlier) and
            # chunk-grouped so mm1's first chunks can start before all of
            # A-gen is drained.
            A_sb = gp.tile([P, NC_I * R], f16)
            for cg in range(4):
                psA = psg_pool.tile([P, SBW], f32, tag="psg", name=f"psA{cg}")
                for cc in range(8):
                    c = cg * 8 + cc
                    for h in range(2):
                        nc.tensor.matmul(
                            psA[:, cc * R : (cc + 1) * R],
                            wap_t[:, h * IN + c * P : h * IN + (c + 1) * P],
                            G[:, h * FD : (h + 1) * FD],
                            start=(h == 0), stop=(h == 1),
                        )
                nc.scalar.copy(
                    A_sb[:, cg * 8 * R : (cg + 1) * 8 * R], psA[:, 0 : 8 * R]
                )

            # ---- B-gen: B_sb [16, 4096] (fp16), h-accumulated in PSUM ----
            B_sb = gp.tile([FD, OUT], f16)
            for ob in range(NOB):
                psB = psg_pool.tile([P, SBW], f32, tag="psg", name=f"psB{ob}")
                for h in range(2):
                    nc.tensor.matmul(
                        psB[0:FD, :],
                        G[:, h * FD : (h + 1) * FD],
                        wbp_t[:, h * OUT + ob * 512 : h * OUT + (ob + 1) * 512],
                        start=(h == 0), stop=(h == 1),
                    )
                nc.vector.tensor_copy(B_sb[:, ob * 512 : (ob + 1) * 512], psB[0:FD, :])

            # ---- main loop: 4 seq quarters of 512, mm2(q) overlaps mm1(q+1).
            # x is host-packed as [q][cg][p][cc][j]: each [128, 2048] DMA tile
            # carries 4 i-chunks of ONE quarter with 4 KB contiguous lines --
            # max DMA efficiency AND quarter-granular psxa completion.
            for q in range(4):
                psxa = psxa_pool.tile([FD, SBW], f32, tag="psxa", name=f"psxa_{q}")
                for cg in range(8):
                    xt_c = xpool.tile([P, 4 * SBW], f16, tag="xnat")
                    row0 = (q * 8 + cg) * P
                    nc.sync.dma_start(
                        out=xt_c[:],
                        in_=xt_d[row0 : row0 + P, :],
                    )
                    for cc in range(4):
                        c = cg * 4 + cc
                        nc.tensor.matmul(
                            psxa[:],
                            A_sb[:, c * R : (c + 1) * R],
                            xt_c[:, cc * SBW : (cc + 1) * SBW],
                            start=(c == 0), stop=(c == NC_I - 1),
                        )

                xaT = xapool.tile([FD, SBW], f16, tag="xaT")
                nc.vector.tensor_copy(xaT[:], psxa[:])
                for t in range(4):
                    out_sb = opool.tile([P, OUT], f16, tag="osb")
                    for ob in range(NOB):
                        pso = pso_pool.tile([P, 512], f32, tag="pso")
                        nc.tensor.matmul(
                            pso[:],
                            xaT[:, t * P : (t + 1) * P],
                            B_sb[:, ob * 512 : (ob + 1) * 512],
                            start=True, stop=True,
                        )
                        # PSUM->SBUF drain also applies the deferred
                        # softmax normalization 1/s^2 (GPSIMD can't read
                        # PSUM, so split over ACT and DVE)
                        dst = out_sb[:, ob * 512 : (ob + 1) * 512]
                        if ob % 2 == 0:
                            nc.scalar.activation(dst, pso[:], ACTF.Copy, scale=rs2b[:])
                        else:
                            nc.vector.tensor_scalar(
                                dst, pso[:], rs2b[:], None, op0=ALU.mult
                            )
                    srow = (q * 4 + t) * P
                    nc.scalar.dma_start(
                        out=y_d[srow : srow + P, :],
                        in_=out_sb[:],
                    )

    nc.compile()
    return nc


def host_prep(inputs):
    """Build per-core and shared input arrays from the full problem inputs."""
    x = np.asarray(inputs["x"], np.float32)
    ctr = np.ascontiguousarray(np.asarray(inputs["ctr_hidden_states"], np.float32))
    gam = np.ascontiguousarray(
        np.tile(np.asarray(inputs["ln_gamma"], np.float32)[None, :], (BS, 1))
    )
    bet = np.ascontiguousarray(
        np.tile(np.asarray(inputs["ln_beta"], np.float32)[None, :], (BS, 1))
    )
    W1 = np.asarray(inputs["W1"], np.float32)
    w1t = np.ascontiguousarray(
        W1.T.reshape(2, P, CTR_HID).transpose(1, 0, 2).reshape(P, 2 * CTR_HID)
    )
    b1 = np.ascontiguousarray(np.asarray(inputs["b1"], np.float32).reshape(CTR_HID, 1))
    w2t = np.ascontiguousarray(np.asarray(inputs["W2"], np.float32).T)
    b2 = np.ascontiguousarray(np.asarray(inputs["b2"], np.float32).reshape(FD, 1))
    Wa = np.asarray(inputs["Wa"], np.float32)
    WaP = Wa.reshape(R, IN, FD).transpose(0, 2, 1).reshape(R * FD, IN)
    wap = np.ascontiguousarray(
        WaP.reshape(2, P, IN).transpose(1, 0, 2).reshape(P, 2 * IN)
    ).astype(np.float16)
    Wb = np.asarray(inputs["Wb"], np.float32) * SCALING
    WbP = Wb.reshape(R, OUT, FD).transpose(0, 2, 1).reshape(R * FD, OUT)
    wbp = np.ascontiguousarray(
        WbP.reshape(2, P, OUT).transpose(1, 0, 2).reshape(P, 2 * OUT)
    ).astype(np.float16)

    shared = dict(
        ctr=ctr, gam=gam, bet=bet, w1t=w1t, b1=b1, w2t=w2t, b2=b2, wap=wap, wbp=wbp
    )
    in_maps = []
    for c in range(BS):
        onehot = np.zeros((BS,), np.float32)
        onehot[c] = 1.0
        sel = np.ascontiguousarray(np.tile(onehot[None, :], (R, 1)))
        m = dict(shared)
        m["sel"] = sel
        # xT [4096, 2048] repacked so each row of the uploaded 2D tensor is a
        # 4 KB line holding (quarter q, chunk-group cg, partition p, cc, j):
        # row (q*8+cg)*128+p, giving [128, 2048] DMA tiles that carry 4
        # i-chunks of one seq-quarter.
        xt = np.asarray(x[c], np.float16).T  # [4096, 2048]
        xq = xt.reshape(8, 4, P, 4, SBW).transpose(3, 0, 2, 1, 4)
        m["xt"] = np.ascontiguousarray(xq).reshape(IN, SEQ)
        in_maps.append(m)
    return in_maps


def get_compiled():
    global _COMPILED
    if _COMPILED is None:
        _COMPILED = build_program()
    return _COMPILED


def run(inputs, trace=False):
    from concourse.bass_utils import run_bass_kernel_spmd

    nc = get_compiled()
    in_maps = host_prep(inputs)
    res = run_bass_kernel_spmd(nc, in_maps, list(range(BS)), trace=trace)
    out = np.stack(
        [np.asarray(res.results[c]["y"], np.float32) for c in range(BS)], axis=0
    )
    return out, res


def kernel(**inputs) -> np.ndarray:
    out, _ = run(inputs, trace=False)
    return out
